# revision 1
# baseline (speedup 1.0000x reference)
"""Cross-attention (single-head) Trainium2 kernel, batch-parallel over 8 NeuronCores.

Reference computation (per batch b):
    q = T_b @ Wq.T            [LQ, D]
    k = S_b @ Wk.T            [LK, D]
    v = S_b @ Wv.T            [LK, D]
    attn = softmax(q @ k.T / sqrt(D))      [LQ, LK]
    out  = (attn @ v) @ Wo.T               [LQ, D]

Device-side layout: everything is kept "feature-on-partition" (transposed),
so every matmul contracts over the partition dim with no on-device transposes:
    qT[e, lq]  = Wq @ T_b.T          (lhsT = Wq.T,  rhs = T_b.T)
    kT[e, lk]  = Wk @ S_b.T          (lhsT = Wk.T,  rhs = S_b.T)
    v [lk, e]  = S_b @ Wv.T          (lhsT = S_b.T, rhs = Wv.T)
    sT[lk, lq] = kT.T @ qT  -> exp(sT/32) (no max-subtraction; |scores/32| ~ 1.5)
    o1[e, lq]  = v.T @ expT, softmax denominator summed on DVE + one
                 partition-reduce matmul, normalization by a reciprocal row
                 broadcast over partitions via a K=1 ones outer-product matmul
    outT[f,lq] = Wo @ o1             (lhsT = Wo.T,  rhs = o1)
Host transposes outT back. Matmuls in bf16 (fp32 runs at 1/4 rate on PE),
accumulation in fp32 PSUM, softmax denominator/normalization in fp32.
Projection-phase loops order consecutive matmuls to share each stationary
operand across both moving tiles (fewer effective weight loads, HW-measured
~12% faster).

B=8 batches -> one batch per core, SPMD, no collectives.
"""

import numpy as np
import ml_dtypes

import concourse.bass as bass
import concourse.mybir as mybir
import concourse.tile as tile
from concourse import bacc
from concourse.bass_utils import run_bass_kernel_spmd

B, LQ, LK, D = 8, 1024, 4096, 1024
P = 128
N_CORES = 8
SCALE = float(D) ** -0.5  # 1/32
BF16 = mybir.dt.bfloat16
F32 = mybir.dt.float32

ED = D // P    # 8   e/d/f chunks of 128
CK = LK // P   # 32  lk chunks of 128
LKT = LK // 512  # 8 lk tiles of 512
import os as _os

ATT_N = int(_os.environ.get("KRN_ATT_N", "512"))  # lq tile width, attention phase
N_ATT = LQ // ATT_N
EXP_BUFS = CK + (1 if ATT_N == 512 else 2)  # exp ring: CK live per lq-tile + slack
ROWSUM_DVE = bool(int(_os.environ.get("KRN_ROWSUM_DVE", "1")))
LQT2 = LQ // 512  # 2 lq tiles of 512 (projection phases)

_PROG = None
LAST_RESULT = None


def _body(nc, tc, tT_d, sT_d, wq_d, wk_d, wv_d, wo_d, outT_d):
    Exp = mybir.ActivationFunctionType.Exp

    with (
        tc.tile_pool(name="misc", bufs=1) as misc,
        tc.tile_pool(name="qTp", bufs=1) as qTp,
        tc.tile_pool(name="kTp", bufs=1) as kTp,
        tc.tile_pool(name="vvp", bufs=1) as vvp,
    ):
        ones_col = misc.tile([P, 8], BF16, tag="ones_col", name="ones_col")
        ones_colf = misc.tile([P, 8], F32, tag="ones_colf", name="ones_colf")
        ones_row = misc.tile([1, P], F32, tag="ones_row", name="ones_row")
        rsum = misc.tile([1, LQ], F32, tag="rsum", name="rsum")
        recip = misc.tile([1, LQ], F32, tag="recip", name="recip")
        nc.gpsimd.memset(ones_col[:], 1.0)
        nc.gpsimd.memset(ones_colf[:], 1.0)
        nc.gpsimd.memset(ones_row[:], 1.0)

        qT = [qTp.tile([P, LQ], BF16, tag=f"q{e}", name=f"qT{e}") for e in range(ED)]
        kT = [kTp.tile([P, LK], BF16, tag=f"k{e}", name=f"kT{e}") for e in range(ED)]
        vv = [vvp.tile([P, D], BF16, tag=f"v{c}", name=f"vv{c}") for c in range(CK)]

        # ---------------- phase 1a: qT[e, lq] = Wq @ T^T ----------------
        with (
            tc.tile_pool(name="p1a", bufs=1) as p1a,
            tc.tile_pool(name="ps1a", bufs=6, space="PSUM") as ps1a,
        ):
            wq = [p1a.tile([P, D], BF16, tag=f"wq{d}", name=f"wq{d}") for d in range(ED)]
            tT = [p1a.tile([P, LQ], BF16, tag=f"tT{d}", name=f"tT{d}") for d in range(ED)]
            for d in range(ED):
                nc.sync.dma_start(wq[d][:], wq_d[d * P:(d + 1) * P, :])
                nc.sync.dma_start(tT[d][:], tT_d[d * P:(d + 1) * P, :])
            for e in range(ED):
                # both lq halves per (d, e) stationary: consecutive matmuls
                # share one weight load
                pss = [
                    ps1a.tile([P, 512], F32, tag="ps", name=f"ps1a_{e}_{n}")
                    for n in range(LQT2)
                ]
                for d in range(ED):
                    for n in range(LQT2):
                        nc.tensor.matmul(
                            pss[n][:],
                            wq[d][:, e * P:(e + 1) * P],
                            tT[d][:, n * 512:(n + 1) * 512],
                            start=(d == 0),
                            stop=(d == ED - 1),
                        )
                for n in range(LQT2):
                    nc.vector.tensor_copy(qT[e][:, n * 512:(n + 1) * 512], pss[n][:])

        # ------- phase 1b: kT[e, lk] = Wk @ S^T ; v[lk, e] = S @ Wv^T -------
        with (
            tc.tile_pool(name="p1b", bufs=1) as p1b,
            tc.tile_pool(name="sst", bufs=20) as sstp,
            tc.tile_pool(name="ps1b", bufs=6, space="PSUM") as ps1b,
        ):
            wk = [p1b.tile([P, D], BF16, tag=f"wk{d}", name=f"wk{d}") for d in range(ED)]
            wv = [p1b.tile([P, D], BF16, tag=f"wv{d}", name=f"wv{d}") for d in range(ED)]
            for d in range(ED):
                nc.sync.dma_start(wk[d][:], wk_d[d * P:(d + 1) * P, :])
                nc.sync.dma_start(wv[d][:], wv_d[d * P:(d + 1) * P, :])
            for lkp in range(LKT // 2):  # process lk tiles in pairs so the
                sts = []                  # wk stationaries serve two rhs tiles
                for m in range(2):
                    lkt = lkp * 2 + m
                    st = []
                    for d in range(ED):
                        s = sstp.tile([P, 512], BF16, tag="st", name=f"st{lkt}_{d}")
                        nc.sync.dma_start(s[:], sT_d[lkt, d * P:(d + 1) * P, :])
                        st.append(s)
                    sts.append(st)
                for e in range(ED):
                    pss = [
                        ps1b.tile([P, 512], F32, tag="ps", name=f"psk{lkp}_{e}_{m}")
                        for m in range(2)
                    ]
                    for d in range(ED):
                        for m in range(2):
                            nc.tensor.matmul(
                                pss[m][:],
                                wk[d][:, e * P:(e + 1) * P],
                                sts[m][d][:],
                                start=(d == 0),
                                stop=(d == ED - 1),
                            )
                    for m in range(2):
                        lkt = lkp * 2 + m
                        nc.vector.tensor_copy(
                            kT[e][:, lkt * 512:(lkt + 1) * 512], pss[m][:]
                        )
                for m in range(2):
                    lkt = lkp * 2 + m
                    st = sts[m]
                    for j in range(4):
                        c = lkt * 4 + j
                        pss = [
                            ps1b.tile([P, 512], F32, tag="ps", name=f"psv{c}_{n2}")
                            for n2 in range(2)
                        ]
                        for d in range(ED):
                            for n2 in range(2):
                                nc.tensor.matmul(
                                    pss[n2][:],
                                    st[d][:, j * P:(j + 1) * P],
                                    wv[d][:, n2 * 512:(n2 + 1) * 512],
                                    start=(d == 0),
                                    stop=(d == ED - 1),
                                )
                        for n2 in range(2):
                            nc.vector.tensor_copy(
                                vv[c][:, n2 * 512:(n2 + 1) * 512], pss[n2][:]
                            )

        # ---------------- phase 2: attention ----------------
        with tc.tile_pool(name="o1p", bufs=1) as o1p:
            o1 = [o1p.tile([P, LQ], BF16, tag=f"o1_{e}", name=f"o1_{e}") for e in range(ED)]
            with (
                tc.tile_pool(name="p2", bufs=1) as p2,
                tc.tile_pool(name="expp", bufs=EXP_BUFS) as expp,
                tc.tile_pool(name="ps_sc", bufs=3, space="PSUM") as ps_sc,
                tc.tile_pool(name="ps_pv", bufs=3, space="PSUM") as ps_pv,
                tc.tile_pool(name="ps_misc", bufs=1, space="PSUM") as ps_misc,
            ):
                for t in range(N_ATT):
                    lq_sl = slice(t * ATT_N, (t + 1) * ATT_N)
                    # scores^T [lk, lq] in psum, -> exp(scale*scores) bf16 in sbuf
                    exp_tiles = []
                    for c in range(CK):
                        ps = ps_sc.tile([P, ATT_N], F32, tag="sc", name=f"sc{t}_{c}")
                        for e in range(ED):
                            nc.tensor.matmul(
                                ps[:],
                                kT[e][:, c * P:(c + 1) * P],
                                qT[e][:, lq_sl],
                                start=(e == 0),
                                stop=(e == ED - 1),
                            )
                        et = expp.tile([P, ATT_N], BF16, tag="exp", name=f"exp{t}_{c}")
                        nc.scalar.activation(et[:], ps[:], Exp, scale=SCALE)
                        exp_tiles.append(et)
                    # softmax denominator: sum exp over lk
                    psr = ps_misc.tile([1, ATT_N], F32, tag="rs", name=f"rs{t}")
                    racc = None
                    if ROWSUM_DVE:
                        # accumulate chunk-sums on DVE; PE partition-reduce is
                        # interleaved with PV below so it never stalls PE
                        racc = p2.tile([P, ATT_N], F32, tag="racc", bufs=1, name=f"racc{t}")
                        nc.vector.tensor_add(racc[:], exp_tiles[0][:], exp_tiles[1][:])
                        for c in range(2, CK):
                            nc.vector.tensor_add(racc[:], racc[:], exp_tiles[c][:])
                    else:
                        # ones-column lhsT: PE pass over all exp tiles
                        for c in range(CK):
                            nc.tensor.matmul(
                                psr[:], ones_col[:, 0:1], exp_tiles[c][:],
                                start=(c == 0), stop=(c == CK - 1),
                            )
                        nc.vector.tensor_copy(rsum[0:1, lq_sl], psr[:])
                        nc.vector.reciprocal(recip[0:1, lq_sl], rsum[0:1, lq_sl])
                        psR0 = ps_misc.tile([P, ATT_N], F32, tag="R", name=f"psR{t}")
                        nc.tensor.matmul(
                            psR0[:], ones_row[0:1, :], recip[0:1, lq_sl],
                            start=True, stop=True,
                        )
                        Rt0 = p2.tile([P, ATT_N], F32, tag="R", bufs=2, name=f"Ra{t}")
                        nc.vector.tensor_copy(Rt0[:], psR0[:])
                    # o1[e, lq] = (v^T @ expT) * R.  In the DVE-rowsum path the
                    # psr/R matmuls are slotted between PV passes 0 and 1 so the
                    # DVE rsum->recip chain overlaps PE work instead of stalling it.
                    Rt = None if ROWSUM_DVE else Rt0
                    pv_ps = []
                    for e in range(ED):
                        ps = ps_pv.tile([P, ATT_N], F32, tag="pv", name=f"pv{t}_{e}")
                        for c in range(CK):
                            nc.tensor.matmul(
                                ps[:],
                                vv[c][:, e * P:(e + 1) * P],
                                exp_tiles[c][:],
                                start=(c == 0),
                                stop=(c == CK - 1),
                            )
                        pv_ps.append(ps)
                        if ROWSUM_DVE and e == 0:
                            nc.tensor.matmul(
                                psr[:], ones_colf[:, 0:1], racc[:], start=True, stop=True
                            )
                            nc.vector.tensor_copy(rsum[0:1, lq_sl], psr[:])
                            nc.vector.reciprocal(recip[0:1, lq_sl], rsum[0:1, lq_sl])
                        if ROWSUM_DVE and e == 1:
                            psR = ps_misc.tile([P, ATT_N], F32, tag="R", name=f"psR{t}")
                            nc.tensor.matmul(
                                psR[:], ones_row[0:1, :], recip[0:1, lq_sl],
                                start=True, stop=True,
                            )
                            Rt = p2.tile([P, ATT_N], F32, tag="R", bufs=2, name=f"R{t}")
                            nc.vector.tensor_copy(Rt[:], psR[:])
                            nc.vector.tensor_mul(o1[0][:, lq_sl], pv_ps[0][:], Rt[:])
                        if Rt is not None and (e >= 1 or not ROWSUM_DVE):
                            nc.vector.tensor_mul(o1[e][:, lq_sl], ps[:], Rt[:])

            # ---------------- phase 3: outT[f, lq] = Wo @ o1 ----------------
            with (
                tc.tile_pool(name="p3", bufs=1) as p3,
                tc.tile_pool(name="finp", bufs=4) as finp,
                tc.tile_pool(name="ps3", bufs=4, space="PSUM") as ps3,
            ):
                wo = [p3.tile([P, D], BF16, tag=f"wo{e}", name=f"wo{e}") for e in range(ED)]
                for e in range(ED):
                    nc.sync.dma_start(wo[e][:], wo_d[e * P:(e + 1) * P, :])
                for f in range(ED):
                    pss = [
                        ps3.tile([P, 512], F32, tag="ps", name=f"ps3_{n}_{f}")
                        for n in range(LQT2)
                    ]
                    for e in range(ED):
                        for n in range(LQT2):
                            nc.tensor.matmul(
                                pss[n][:],
                                wo[e][:, f * P:(f + 1) * P],
                                o1[e][:, n * 512:(n + 1) * 512],
                                start=(e == 0),
                                stop=(e == ED - 1),
                            )
                    for n in range(LQT2):
                        ft = finp.tile([P, 512], F32, tag="fin", name=f"fin{n}_{f}")
                        nc.vector.tensor_copy(ft[:], pss[n][:])
                        nc.sync.dma_start(
                            outT_d[f * P:(f + 1) * P, n * 512:(n + 1) * 512], ft[:]
                        )


def build_program(reps: int = 1):
    nc = bacc.Bacc("TRN2", target_bir_lowering=False, debug=False)
    tT_d = nc.dram_tensor("tT", [D, LQ], BF16, kind="ExternalInput")
    sT_d = nc.dram_tensor("sT", [LKT, D, 512], BF16, kind="ExternalInput")
    wq_d = nc.dram_tensor("wqT", [D, D], BF16, kind="ExternalInput")
    wk_d = nc.dram_tensor("wkT", [D, D], BF16, kind="ExternalInput")
    wv_d = nc.dram_tensor("wvT", [D, D], BF16, kind="ExternalInput")
    wo_d = nc.dram_tensor("woT", [D, D], BF16, kind="ExternalInput")
    outT_d = nc.dram_tensor("outT", [D, LQ], F32, kind="ExternalOutput")

    with tile.TileContext(nc) as tc:
        # reps>1 is the benchmark variant: straight-line repetition of the
        # whole computation so per-rep time can be measured as a slope
        # (cancels the ~100ms host/axon RPC latency and its jitter)
        for _ in range(reps):
            _body(nc, tc, tT_d, sT_d, wq_d, wk_d, wv_d, wo_d, outT_d)
    nc.compile()
    return nc


def _get_prog():
    global _PROG
    if _PROG is None:
        _PROG = build_program()
    return _PROG


def make_in_maps(T, S, Wq, Wk, Wv, Wo):
    bf = ml_dtypes.bfloat16
    wqT = np.ascontiguousarray(Wq.T).astype(bf)
    wkT = np.ascontiguousarray(Wk.T).astype(bf)
    wvT = np.ascontiguousarray(Wv.T).astype(bf)
    woT = np.ascontiguousarray(Wo.T).astype(bf)
    in_maps = []
    for b in range(T.shape[0]):
        tT = np.ascontiguousarray(T[b].T).astype(bf)  # [D, LQ]
        sT = np.ascontiguousarray(S[b].T).astype(bf)  # [D, LK]
        sTb = np.ascontiguousarray(
            sT.reshape(D, LKT, 512).transpose(1, 0, 2)
        )  # [LKT, D, 512] so each streamed tile is contiguous
        in_maps.append(
            {"tT": tT, "sT": sTb, "wqT": wqT, "wkT": wkT, "wvT": wvT, "woT": woT}
        )
    return in_maps


def kernel(**inputs):
    global LAST_RESULT
    T = np.asarray(inputs["T"], dtype=np.float32)
    S = np.asarray(inputs["S"], dtype=np.float32)
    Wq = np.asarray(inputs["Wq"], dtype=np.float32)
    Wk = np.asarray(inputs["Wk"], dtype=np.float32)
    Wv = np.asarray(inputs["Wv"], dtype=np.float32)
    Wo = np.asarray(inputs["Wo"], dtype=np.float32)

    nc = _get_prog()
    in_maps = make_in_maps(T, S, Wq, Wk, Wv, Wo)
    # NTFF profiling hook (antenv.axon_hooks) is absent in this environment;
    # a stray BASS_TRACE=1 would crash the axon path, so pin tracing off.
    _os.environ["BASS_NEVER_TRACE"] = "1"
    res = run_bass_kernel_spmd(nc, in_maps, list(range(N_CORES)))
    LAST_RESULT = res
    out = np.stack(
        [res.results[b]["outT"].T for b in range(N_CORES)], axis=0
    ).astype(np.float32)
    return out



# revision 2
# speedup vs baseline: 1.8821x; 1.8821x over previous
"""Cross-attention (single-head) Trainium2 kernel, batch-parallel over 8 NeuronCores.

Reference computation (per batch b):
    q = T_b @ Wq.T            [LQ, D]
    k = S_b @ Wk.T            [LK, D]
    v = S_b @ Wv.T            [LK, D]
    attn = softmax(q @ k.T / sqrt(D))      [LQ, LK]
    out  = (attn @ v) @ Wo.T               [LQ, D]

Algebraic folding (host-side, fp64): the K and V projections are folded
into their neighbours, eliminating 8.6 of 19.3 GMAC per core:
    scores = q @ k.T = T @ (Wq.T Wk) @ S.T          -> Wqk := Wq.T @ Wk
    out    = (attn @ v) @ Wo.T = (attn @ S) @ (Wo Wv).T -> Wvo := Wo @ Wv
so the device computes
    q'  = T_b @ Wqk                     [LQ, D]   (phase 1)
    attn = softmax(q' @ S.T / sqrt(D))  [LQ, LK]  (phase 2a, S.T streamed)
    o1  = attn @ S                      [LQ, D]   (phase 2b, S streamed)
    out = o1 @ Wvo.T                    [LQ, D]   (phase 3)

Device-side layout: everything is kept "feature-on-partition" (transposed),
so every matmul contracts over the partition dim with no on-device transposes:
    qT[e, lq]  = Wqk.T @ T_b.T       (lhsT = Wqk,  rhs = T_b.T)
    kT[e, lk]  = S.T rows            (pure DMA)
    vv[lk, e]  = S rows              (pure DMA)
    sT[lk, lq] = kT.T @ qT  -> exp(sT/32) (no max-subtraction; |scores/32| ~ 1.5)
    o1[e, lq]  = vv.T @ expT, softmax denominator summed on DVE + one
                 partition-reduce matmul, normalization by a reciprocal row
                 broadcast over partitions via a K=1 ones outer-product matmul
    outT[f,lq] = Wvo @ o1            (lhsT = Wvo.T, rhs = o1)
Host transposes outT back. Matmuls in bf16 (fp32 runs at 1/4 rate on PE),
accumulation in fp32 PSUM, softmax denominator/normalization in fp32.

B=8 batches -> one batch per core, SPMD, no collectives.
"""

import numpy as np
import ml_dtypes

import concourse.bass as bass
import concourse.mybir as mybir
import concourse.tile as tile
from concourse import bacc
from concourse.bass_utils import run_bass_kernel_spmd

B, LQ, LK, D = 8, 1024, 4096, 1024
P = 128
N_CORES = 8
SCALE = float(D) ** -0.5  # 1/32
BF16 = mybir.dt.bfloat16
F32 = mybir.dt.float32

ED = D // P    # 8   e/d/f chunks of 128
CK = LK // P   # 32  lk chunks of 128
import os as _os

ATT_N = int(_os.environ.get("KRN_ATT_N", "512"))  # lq tile width, attention phase
N_ATT = LQ // ATT_N
EXP_BUFS = CK + (1 if ATT_N == 512 else 2)  # exp ring: CK live per lq-tile + slack
LQT2 = LQ // 512  # 2 lq tiles of 512 (projection phases)

_PROG = None
LAST_RESULT = None


def _body(nc, tc, tT_d, sT_d, sB_d, wq_d, wo_d, outT_d):
    Exp = mybir.ActivationFunctionType.Exp

    with (
        tc.tile_pool(name="misc", bufs=1) as misc,
        tc.tile_pool(name="qTp", bufs=1) as qTp,
        tc.tile_pool(name="kTp", bufs=1) as kTp,
        tc.tile_pool(name="vvp", bufs=1) as vvp,
    ):
        ones_colf = misc.tile([P, 8], F32, tag="ones_colf", name="ones_colf")
        ones_row = misc.tile([1, P], F32, tag="ones_row", name="ones_row")
        rsum = misc.tile([1, LQ], F32, tag="rsum", name="rsum")
        recip = misc.tile([1, LQ], F32, tag="recip", name="recip")
        nc.gpsimd.memset(ones_colf[:], 1.0)
        nc.gpsimd.memset(ones_row[:], 1.0)

        qT = [qTp.tile([P, LQ], BF16, tag=f"q{e}", name=f"qT{e}") for e in range(ED)]
        kT = [kTp.tile([P, LK], BF16, tag=f"k{e}", name=f"kT{e}") for e in range(ED)]
        vv = [vvp.tile([P, D], BF16, tag=f"v{c}", name=f"vv{c}") for c in range(CK)]

        # ---------------- phase 1: qT[e, lq] = Wqk.T @ T^T ----------------
        with (
            tc.tile_pool(name="p1a", bufs=1) as p1a,
            tc.tile_pool(name="ps1a", bufs=6, space="PSUM") as ps1a,
        ):
            wq = [p1a.tile([P, D], BF16, tag=f"wq{d}", name=f"wq{d}") for d in range(ED)]
            tT = [p1a.tile([P, LQ], BF16, tag=f"tT{d}", name=f"tT{d}") for d in range(ED)]
            for d in range(ED):
                nc.sync.dma_start(wq[d][:], wq_d[d * P:(d + 1) * P, :])
                nc.sync.dma_start(tT[d][:], tT_d[d * P:(d + 1) * P, :])
            # stream S into SBUF behind the phase-1 operands: kT (scores
            # stationaries, needed first) then vv (PV stationaries)
            for e in range(ED):
                nc.sync.dma_start(kT[e][:], sT_d[e * P:(e + 1) * P, :])
            for c in range(CK):
                nc.sync.dma_start(vv[c][:], sB_d[c * P:(c + 1) * P, :])
            for e in range(ED):
                # both lq halves per (d, e) stationary: consecutive matmuls
                # share one weight load
                pss = [
                    ps1a.tile([P, 512], F32, tag="ps", name=f"ps1a_{e}_{n}")
                    for n in range(LQT2)
                ]
                for d in range(ED):
                    for n in range(LQT2):
                        nc.tensor.matmul(
                            pss[n][:],
                            wq[d][:, e * P:(e + 1) * P],
                            tT[d][:, n * 512:(n + 1) * 512],
                            start=(d == 0),
                            stop=(d == ED - 1),
                        )
                for n in range(LQT2):
                    nc.vector.tensor_copy(qT[e][:, n * 512:(n + 1) * 512], pss[n][:])

        # ---------------- phase 2: attention ----------------
        with tc.tile_pool(name="o1p", bufs=1) as o1p:
            o1 = [o1p.tile([P, LQ], BF16, tag=f"o1_{e}", name=f"o1_{e}") for e in range(ED)]
            with (
                tc.tile_pool(name="p2", bufs=1) as p2,
                tc.tile_pool(name="expp", bufs=EXP_BUFS) as expp,
                tc.tile_pool(name="ps_sc", bufs=3, space="PSUM") as ps_sc,
                tc.tile_pool(name="ps_pv", bufs=3, space="PSUM") as ps_pv,
                tc.tile_pool(name="ps_misc", bufs=1, space="PSUM") as ps_misc,
            ):
                for t in range(N_ATT):
                    lq_sl = slice(t * ATT_N, (t + 1) * ATT_N)
                    # scores^T [lk, lq] in psum, -> exp(scale*scores) bf16 in sbuf
                    exp_tiles = []
                    for c in range(CK):
                        ps = ps_sc.tile([P, ATT_N], F32, tag="sc", name=f"sc{t}_{c}")
                        for e in range(ED):
                            nc.tensor.matmul(
                                ps[:],
                                kT[e][:, c * P:(c + 1) * P],
                                qT[e][:, lq_sl],
                                start=(e == 0),
                                stop=(e == ED - 1),
                            )
                        et = expp.tile([P, ATT_N], BF16, tag="exp", name=f"exp{t}_{c}")
                        nc.scalar.activation(et[:], ps[:], Exp, scale=SCALE)
                        exp_tiles.append(et)
                    # softmax denominator: accumulate chunk-sums on DVE; PE
                    # partition-reduce is interleaved with PV below so it
                    # never stalls PE
                    psr = ps_misc.tile([1, ATT_N], F32, tag="rs", name=f"rs{t}")
                    racc = p2.tile([P, ATT_N], F32, tag="racc", bufs=1, name=f"racc{t}")
                    nc.vector.tensor_add(racc[:], exp_tiles[0][:], exp_tiles[1][:])
                    for c in range(2, CK):
                        nc.vector.tensor_add(racc[:], racc[:], exp_tiles[c][:])
                    # o1[e, lq] = (vv^T @ expT) * R.  The psr/R matmuls are
                    # slotted between PV passes 0 and 1 so the DVE
                    # rsum->recip chain overlaps PE work instead of stalling it.
                    Rt = None
                    pv_ps = []
                    for e in range(ED):
                        ps = ps_pv.tile([P, ATT_N], F32, tag="pv", name=f"pv{t}_{e}")
                        for c in range(CK):
                            nc.tensor.matmul(
                                ps[:],
                                vv[c][:, e * P:(e + 1) * P],
                                exp_tiles[c][:],
                                start=(c == 0),
                                stop=(c == CK - 1),
                            )
                        pv_ps.append(ps)
                        if e == 0:
                            nc.tensor.matmul(
                                psr[:], ones_colf[:, 0:1], racc[:], start=True, stop=True
                            )
                            nc.vector.tensor_copy(rsum[0:1, lq_sl], psr[:])
                            nc.vector.reciprocal(recip[0:1, lq_sl], rsum[0:1, lq_sl])
                        if e == 1:
                            psR = ps_misc.tile([P, ATT_N], F32, tag="R", name=f"psR{t}")
                            nc.tensor.matmul(
                                psR[:], ones_row[0:1, :], recip[0:1, lq_sl],
                                start=True, stop=True,
                            )
                            Rt = p2.tile([P, ATT_N], F32, tag="R", bufs=2, name=f"R{t}")
                            nc.vector.tensor_copy(Rt[:], psR[:])
                            nc.vector.tensor_mul(o1[0][:, lq_sl], pv_ps[0][:], Rt[:])
                        if Rt is not None and e >= 1:
                            nc.vector.tensor_mul(o1[e][:, lq_sl], ps[:], Rt[:])

            # ---------------- phase 3: outT[f, lq] = Wvo @ o1 ----------------
            with (
                tc.tile_pool(name="p3", bufs=1) as p3,
                tc.tile_pool(name="finp", bufs=4) as finp,
                tc.tile_pool(name="ps3", bufs=4, space="PSUM") as ps3,
            ):
                wo = [p3.tile([P, D], BF16, tag=f"wo{e}", name=f"wo{e}") for e in range(ED)]
                for e in range(ED):
                    nc.sync.dma_start(wo[e][:], wo_d[e * P:(e + 1) * P, :])
                for f in range(ED):
                    pss = [
                        ps3.tile([P, 512], F32, tag="ps", name=f"ps3_{n}_{f}")
                        for n in range(LQT2)
                    ]
                    for e in range(ED):
                        for n in range(LQT2):
                            nc.tensor.matmul(
                                pss[n][:],
                                wo[e][:, f * P:(f + 1) * P],
                                o1[e][:, n * 512:(n + 1) * 512],
                                start=(e == 0),
                                stop=(e == ED - 1),
                            )
                    for n in range(LQT2):
                        ft = finp.tile([P, 512], F32, tag="fin", name=f"fin{n}_{f}")
                        nc.vector.tensor_copy(ft[:], pss[n][:])
                        nc.sync.dma_start(
                            outT_d[f * P:(f + 1) * P, n * 512:(n + 1) * 512], ft[:]
                        )


def build_program(reps: int = 1):
    nc = bacc.Bacc("TRN2", target_bir_lowering=False, debug=False)
    tT_d = nc.dram_tensor("tT", [D, LQ], BF16, kind="ExternalInput")
    sT_d = nc.dram_tensor("sT", [D, LK], BF16, kind="ExternalInput")
    sB_d = nc.dram_tensor("sB", [LK, D], BF16, kind="ExternalInput")
    wq_d = nc.dram_tensor("wqT", [D, D], BF16, kind="ExternalInput")
    wo_d = nc.dram_tensor("woT", [D, D], BF16, kind="ExternalInput")
    outT_d = nc.dram_tensor("outT", [D, LQ], F32, kind="ExternalOutput")

    with tile.TileContext(nc) as tc:
        # reps>1 is the benchmark variant: straight-line repetition of the
        # whole computation so per-rep time can be measured as a slope
        # (cancels the ~100ms host/axon RPC latency and its jitter)
        for _ in range(reps):
            _body(nc, tc, tT_d, sT_d, sB_d, wq_d, wo_d, outT_d)
    nc.compile()
    return nc


def _get_prog():
    global _PROG
    if _PROG is None:
        _PROG = build_program()
    return _PROG


def make_in_maps(T, S, Wq, Wk, Wv, Wo):
    bf = ml_dtypes.bfloat16
    # host-side projection folding in fp64
    Wqk = (Wq.astype(np.float64).T @ Wk.astype(np.float64)).astype(bf)
    WvoT = (Wo.astype(np.float64) @ Wv.astype(np.float64)).T.astype(bf)
    Wqk = np.ascontiguousarray(Wqk)
    WvoT = np.ascontiguousarray(WvoT)
    in_maps = []
    for b in range(T.shape[0]):
        tT = np.ascontiguousarray(T[b].T).astype(bf)  # [D, LQ]
        sT = np.ascontiguousarray(S[b].T).astype(bf)  # [D, LK]
        sB = np.ascontiguousarray(S[b]).astype(bf)    # [LK, D]
        in_maps.append(
            {"tT": tT, "sT": sT, "sB": sB, "wqT": Wqk, "woT": WvoT}
        )
    return in_maps


def kernel(**inputs):
    global LAST_RESULT
    T = np.asarray(inputs["T"], dtype=np.float32)
    S = np.asarray(inputs["S"], dtype=np.float32)
    Wq = np.asarray(inputs["Wq"], dtype=np.float32)
    Wk = np.asarray(inputs["Wk"], dtype=np.float32)
    Wv = np.asarray(inputs["Wv"], dtype=np.float32)
    Wo = np.asarray(inputs["Wo"], dtype=np.float32)

    nc = _get_prog()
    in_maps = make_in_maps(T, S, Wq, Wk, Wv, Wo)
    # NTFF profiling hook (antenv.axon_hooks) is absent in this environment;
    # a stray BASS_TRACE=1 would crash the axon path, so pin tracing off.
    _os.environ["BASS_NEVER_TRACE"] = "1"
    res = run_bass_kernel_spmd(nc, in_maps, list(range(N_CORES)))
    LAST_RESULT = res
    out = np.stack(
        [res.results[b]["outT"].T for b in range(N_CORES)], axis=0
    ).astype(np.float32)
    return out
